# revision 7
# baseline (speedup 1.0000x reference)
"""Trainium2 Bass kernel for nn_CombinedCriterionAE (retrieval 1-NN + losses).

V2 strategy (8 NeuronCores, SPMD, pred-sharded):
  - preds are sharded across cores (1024/core, 8 tiles of 128); every core
    holds the FULL gt set (32768).  Each core computes exact argmin over the
    full row, so no cross-core argmin machinery is needed — the only
    collective is an 8-byte AllReduce of the two loss partial sums.  This
    removes the baseline's 250us tail (AllGather + 8-way fold + 128
    serialized indirect gathers).
  - s = -dist^2 = 2 p.g - p^2 - g^2 on the PE as a K=24 bf16 matmul
    (fp32 operands split host-side into 3 exact bf16 terms; small
    correction rows first, big rows last) -- s matches the reference's
    fp32 rounding to ~1e-6 so argmin picks track the reference.
  - Per 2048-col PSUM chunk: ACT stages the upper 1024 cols to SBUF (DVE
    cannot read two PSUM operands), one DVE tensor_tensor_scan computes the
    running max of pairs (j, j+1024), chained across the 8 chunks of a
    16384-wide super-chunk.
  - Per super-chunk (2 per tile): ONE ACT sign+accum over the full 8192
    scan outputs against the super-chunk's own max gives the first-occurrence
    winner position; the supermax lands in CMALL, the count in CNTALL.
    Counting against the LOCAL supermax (not a global max) keeps scan
    buffers short-lived and lets the sign pass batch 8 chunks into one
    instruction (amortizes the accumulator-read).
  - After the loop, positions resolve vectorized over all [128, 8, 2]
    candidates at once (no data-dependent addressing): winner super-chunk by
    is_ge (ties -> earlier), pair member by gathering both candidate gt rows
    (16 indirect DMAs) and comparing fp32 dist^2.
"""
import os
import numpy as np
import ml_dtypes

import concourse.bass as bass
import concourse.bacc as bacc
import concourse.mybir as mybir
import concourse.tile as tile
from concourse.bass import IndirectOffsetOnAxis

BF16 = ml_dtypes.bfloat16
DT = mybir.dt
OP = mybir.AluOpType

N_PRED = 8192
L_GT = 32768
NCORES = 8
K_SMALL = 19
K_BIG = 5
NEG_INF = -3.0e38


# ----------------------------------------------------------------------------
# host-side input prep
# ----------------------------------------------------------------------------

def _split3(x):
    x = np.asarray(x, np.float32)
    hi = x.astype(BF16)
    r = x - hi.astype(np.float32)
    mid = r.astype(BF16)
    r2 = r - mid.astype(np.float32)
    lo = r2.astype(BF16)
    return hi, mid, lo


def build_operands(pred_pts, gt_pts):
    """lhsT [24, N] / rhs [24, L] bf16; 19 small rows then 5 big rows."""
    q = 2.0 * np.asarray(pred_pts, np.float32)
    qh, qm, ql = _split3(q.T)
    gh, gm, gl = _split3(np.asarray(gt_pts, np.float32).T)
    g2 = (np.asarray(gt_pts, np.float32) ** 2).sum(1)
    p2 = (np.asarray(pred_pts, np.float32) ** 2).sum(1)
    g2h, g2m, g2l = _split3(g2)
    p2h, p2m, p2l = _split3(p2)
    ones_g = np.ones(gt_pts.shape[0], BF16)
    neg1_p = -np.ones(pred_pts.shape[0], BF16)

    lhs, rhs = [], []

    def add(a, b):
        lhs.append(a)
        rhs.append(b)

    for d in range(3):
        add(qh[d], gm[d]); add(qm[d], gh[d]); add(qm[d], gm[d])
        add(qh[d], gl[d]); add(ql[d], gh[d])
    add(neg1_p, g2m); add(neg1_p, g2l)
    add((-p2m).astype(BF16), ones_g); add((-p2l).astype(BF16), ones_g)
    # big rows
    add(qh[0], gh[0]); add(qh[1], gh[1]); add(qh[2], gh[2])
    add((-p2h).astype(BF16), ones_g); add(neg1_p, g2h)
    return np.ascontiguousarray(np.stack(lhs)), np.ascontiguousarray(np.stack(rhs))


def prep_inputs(pred_feat, gt_data, ncores):
    """Returns the per-core in_map list (preds sharded, gt full)."""
    pred_feat = np.asarray(pred_feat, np.float32)
    gt_data = np.asarray(gt_data, np.float32)
    n_pred = pred_feat.shape[0]
    npc = n_pred // ncores          # preds per core
    nt = npc // 128                 # pred tiles per core
    pred_pts = pred_feat[:, :3]
    pred_nrm = pred_feat[:, 3:]
    lhsT, rhs = build_operands(pred_pts, gt_data[:, :3])

    in_maps = []
    for c in range(ncores):
        sl = slice(npc * c, npc * (c + 1))
        pp = np.ascontiguousarray(
            pred_pts[sl].reshape(nt, 128, 3).transpose(1, 0, 2))
        pn = np.ascontiguousarray(
            pred_nrm[sl].reshape(nt, 128, 3).transpose(1, 0, 2))
        in_maps.append({
            "lhs": np.ascontiguousarray(lhsT[:, sl]),
            "rhs": rhs,
            "pp": pp,
            "pn": pn,
            "gtf": gt_data,
        })
    return in_maps


# ----------------------------------------------------------------------------
# device program
# ----------------------------------------------------------------------------

def build_nc(n_pred=N_PRED, l_gt=L_GT, ncores=NCORES):
    npc = n_pred // ncores
    nt = npc // 128                 # 8 pred tiles per core
    nsc = 2                         # super-chunks per tile
    nch = l_gt // (nsc * 2048)      # 8 chunks per super-chunk
    sw = nch * 1024                 # scan width per super-chunk (8192)
    assert nch * nsc * 2048 == l_gt

    nc = bacc.Bacc("TRN2", target_bir_lowering=False, debug=False,
                   num_devices=ncores)

    kk = K_SMALL + K_BIG
    lhs_d = nc.dram_tensor("lhs", [kk, npc], DT.bfloat16, kind="ExternalInput")
    rhs_d = nc.dram_tensor("rhs", [kk, l_gt], DT.bfloat16, kind="ExternalInput")
    pp_d = nc.dram_tensor("pp", [128, nt, 3], DT.float32, kind="ExternalInput")
    pn_d = nc.dram_tensor("pn", [128, nt, 3], DT.float32, kind="ExternalInput")
    gtf_d = nc.dram_tensor("gtf", [l_gt, 6], DT.float32, kind="ExternalInput")
    out_d = nc.dram_tensor("out", [1, 1], DT.float32, kind="ExternalOutput")

    with tile.TileContext(nc) as tc:
        with (
            tc.tile_pool(name="persist", bufs=1) as pers,
            tc.tile_pool(name="hpool", bufs=3) as hpool,
            tc.tile_pool(name="scnp", bufs=2) as scnp,
            tc.tile_pool(name="mkp", bufs=2) as mkp,
            tc.tile_pool(name="dram", bufs=1, space="DRAM") as dram,
        ):
            # ---- persistent SBUF loads -------------------------------------
            LHS = pers.tile([kk, npc], DT.bfloat16)
            RHS = pers.tile([kk, l_gt], DT.bfloat16)
            PP = pers.tile([128, nt, 3], DT.float32)
            PN = pers.tile([128, nt, 3], DT.float32)
            nc.sync.dma_start(LHS[:], lhs_d[:])
            # split the RHS load so the first matmuls only wait on slice 0
            for ksl in range(16):
                sl = slice(2048 * ksl, 2048 * (ksl + 1))
                nc.sync.dma_start(RHS[:, sl], rhs_d[:, sl])
            nc.sync.dma_start(PP[:], pp_d[:])
            nc.sync.dma_start(PN[:], pn_d[:])

            # per-(tile, super-chunk) results: columns s*nt+i; counts come in
            # 8 pieces of 1024 per super-chunk.  A full piece counts 1024
            # (prefix below the max throughout), the winner piece counts the
            # in-piece offset, later pieces count 0 -- so sum(pieces) = p and
            # count(pieces == 1024) = the winner chunk c*.
            CMALL = pers.tile([128, nsc * nt], DT.float32)
            CNT8 = pers.tile([128, nsc * nt, nch], DT.float32)

            # ---- main loop: s-matmul, ACT staging, pairwise-max scan -------
            # The sign+count of super-chunk k is split into 1024-wide pieces
            # emitted between the NEXT super-chunk's stage copies, so ACT's
            # in-order queue never delays a stage copy by more than ~1.2us (a
            # big sign blob would stall the scans behind the staging).
            def emit_piece(pend, j):
                SCNp, smax_p, kp = pend
                MK = mkp.tile([128, 1024], DT.float16, tag="MK")
                nc.scalar.activation(
                    out=MK[:], in_=SCNp[:, 1024 * j:1024 * (j + 1)],
                    func=mybir.ActivationFunctionType.Sign,
                    bias=smax_p, scale=-1.0,
                    accum_out=CNT8[:, kp, j:j + 1],
                )

            pending = None
            with tc.tile_pool(name="spsum", bufs=2, space="PSUM") as spsum:
                for i in range(nt):
                    for s in range(nsc):
                        SCN = scnp.tile([128, sw], DT.float32, tag="SCN")
                        for c in range(nch):
                            P = spsum.tile([128, 2048], DT.float32, tag="P")
                            for t in range(4):
                                col = 2048 * (nch * s + c) + 512 * t
                                nc.tensor.matmul(
                                    P[:, 512 * t:512 * (t + 1)],
                                    LHS[:, 128 * i:128 * (i + 1)],
                                    RHS[:, col:col + 512],
                                    start=True, stop=True,
                                )
                            HB = hpool.tile([128, 1024], DT.float32, tag="HB")
                            nc.scalar.activation(
                                out=HB[:], in_=P[:, 1024:2048],
                                func=mybir.ActivationFunctionType.Copy,
                            )
                            nc.vector.tensor_tensor_scan(
                                out=SCN[:, 1024 * c:1024 * (c + 1)],
                                data0=P[:, 0:1024],
                                data1=HB[:],
                                initial=(NEG_INF if c == 0
                                         else SCN[:, 1024 * c - 1:1024 * c]),
                                op0=OP.max,
                                op1=OP.max,
                            )
                            if pending is not None:
                                emit_piece(pending, c)
                        k = s * nt + i
                        smax_ap = SCN[:, sw - 1:sw]
                        nc.vector.tensor_copy(out=CMALL[:, k:k + 1], in_=smax_ap)
                        pending = (SCN, smax_ap, k)
                # last super-chunk's pieces
                for j in range(nch):
                    emit_piece(pending, j)

            # counts -> p (sum of pieces) and 1024*c* (count of full pieces)
            CNTALL = pers.tile([128, nsc * nt], DT.float32)
            nc.vector.tensor_reduce(out=CNTALL[:], in_=CNT8[:],
                                    axis=mybir.AxisListType.X, op=OP.add)
            GEQ = pers.tile([128, nsc * nt, nch], DT.float32)
            nc.vector.tensor_scalar(out=GEQ[:], in0=CNT8[:],
                                    scalar1=1023.5, scalar2=1024.0,
                                    op0=OP.is_ge, op1=OP.mult)
            CQ = pers.tile([128, nsc * nt], DT.float32)
            nc.vector.tensor_reduce(out=CQ[:], in_=GEQ[:],
                                    axis=mybir.AxisListType.X, op=OP.add)
            # col16 = p + 1024*c* + 16384*s
            COL16 = pers.tile([128, nsc * nt], DT.float32)
            SBASE = pers.tile([128, nsc * nt], DT.float32)
            nc.vector.memset(SBASE[:, 0:nt], 0.0)
            nc.vector.memset(SBASE[:, nt:2 * nt], float(sw * 2))
            nc.vector.tensor_tensor(out=COL16[:], in0=CNTALL[:], in1=CQ[:], op=OP.add)
            nc.vector.tensor_tensor(out=COL16[:], in0=COL16[:], in1=SBASE[:], op=OP.add)

            # ---- pick the winning super-chunk per pred ---------------------
            CM0 = CMALL[:, 0:nt]
            CM1 = CMALL[:, nt:2 * nt]
            W0 = pers.tile([128, nt], DT.uint8)
            nc.vector.tensor_tensor(out=W0[:], in0=CM0, in1=CM1, op=OP.is_ge)
            L0G = pers.tile([128, nt], DT.float32)
            nc.vector.select(out=L0G[:], mask=W0[:], on_true=COL16[:, 0:nt],
                             on_false=COL16[:, nt:2 * nt])
            L1G = pers.tile([128, nt], DT.float32)
            nc.vector.tensor_scalar(out=L1G[:], in0=L0G[:], scalar1=1024.0,
                                    scalar2=None, op0=OP.add)

            # ---- gather both candidates, resolve the pair member -----------
            I0 = pers.tile([128, nt], DT.int32)
            I1 = pers.tile([128, nt], DT.int32)
            nc.vector.tensor_copy(out=I0[:], in_=L0G[:])
            nc.vector.tensor_copy(out=I1[:], in_=L1G[:])
            G0 = pers.tile([128, nt, 6], DT.float32)
            G1 = pers.tile([128, nt, 6], DT.float32)
            # HW supports one offset per partition per indirect DMA, so
            # gather tile-by-tile.
            for i in range(nt):
                nc.gpsimd.indirect_dma_start(
                    out=G0[:, i, :], out_offset=None, in_=gtf_d[:],
                    in_offset=IndirectOffsetOnAxis(ap=I0[:, i:i + 1], axis=0),
                )
                nc.gpsimd.indirect_dma_start(
                    out=G1[:, i, :], out_offset=None, in_=gtf_d[:],
                    in_offset=IndirectOffsetOnAxis(ap=I1[:, i:i + 1], axis=0),
                )
            DF = pers.tile([128, nt, 3], DT.float32)
            SQ = pers.tile([128, nt, 3], DT.float32)
            D0 = pers.tile([128, nt], DT.float32)
            D1 = pers.tile([128, nt], DT.float32)
            nc.vector.tensor_tensor(out=DF[:], in0=PP[:], in1=G0[:, :, 0:3], op=OP.subtract)
            nc.vector.tensor_tensor(out=SQ[:], in0=DF[:], in1=DF[:], op=OP.mult)
            nc.vector.tensor_reduce(out=D0[:], in_=SQ[:], axis=mybir.AxisListType.X, op=OP.add)
            nc.vector.tensor_tensor(out=DF[:], in0=PP[:], in1=G1[:, :, 0:3], op=OP.subtract)
            nc.vector.tensor_tensor(out=SQ[:], in0=DF[:], in1=DF[:], op=OP.mult)
            nc.vector.tensor_reduce(out=D1[:], in_=SQ[:], axis=mybir.AxisListType.X, op=OP.add)
            MEM = pers.tile([128, nt], DT.uint8)
            nc.vector.tensor_tensor(out=MEM[:], in0=D1[:], in1=D0[:], op=OP.is_ge)
            MATCH = pers.tile([128, nt, 6], DT.float32)
            for d in range(6):
                nc.vector.select(out=MATCH[:, :, d], mask=MEM[:],
                                 on_true=G0[:, :, d], on_false=G1[:, :, d])

            # ---- losses (partial sums over this core's preds) --------------
            ILS = pers.tile([128, 1], DT.float32)
            JNK = pers.tile([128, nt, 3], DT.float32)
            nc.vector.tensor_tensor(out=DF[:], in0=PP[:], in1=MATCH[:, :, 0:3], op=OP.subtract)
            nc.vector.tensor_tensor(out=JNK[:], in0=DF[:], in1=DF[:], op=OP.mult)
            nc.vector.tensor_reduce(out=ILS[:], in_=JNK[:],
                                    axis=mybir.AxisListType.XY, op=OP.add)

            def normalize(src3, dst3, tagp):
                NSQ = pers.tile([128, nt, 3], DT.float32, tag=f"NSQ{tagp}", name=f"NSQ{tagp}")
                NS = pers.tile([128, nt], DT.float32, tag=f"NS{tagp}", name=f"NS{tagp}")
                nc.vector.tensor_tensor(out=NSQ[:], in0=src3, in1=src3, op=OP.mult)
                nc.vector.tensor_reduce(out=NS[:], in_=NSQ[:], axis=mybir.AxisListType.X, op=OP.add)
                nc.scalar.activation(out=NS[:], in_=NS[:], func=mybir.ActivationFunctionType.Sqrt)
                nc.vector.tensor_scalar(out=NS[:], in0=NS[:], scalar1=1e-4,
                                        scalar2=None, op0=OP.max)
                nc.vector.reciprocal(out=NS[:], in_=NS[:])
                for d in range(3):
                    nc.vector.tensor_tensor(out=dst3[:, :, d], in0=src3[:, :, d],
                                            in1=NS[:], op=OP.mult)

            PNH = pers.tile([128, nt, 3], DT.float32)
            MNH = pers.tile([128, nt, 3], DT.float32)
            normalize(PN[:], PNH, "a")
            normalize(MATCH[:, :, 3:6], MNH, "b")
            CC3 = pers.tile([128, nt, 3], DT.float32)
            CSUM = pers.tile([128, 1], DT.float32)
            nc.vector.tensor_tensor(out=CC3[:], in0=PNH[:], in1=MNH[:], op=OP.mult)
            nc.vector.tensor_reduce(out=CSUM[:], in_=CC3[:],
                                    axis=mybir.AxisListType.XY, op=OP.add)

            # partition-sum via ones-matmul -> [1, 2] partials
            SUM2 = pers.tile([128, 2], DT.float32)
            ONES = pers.tile([128, 1], DT.float32)
            nc.vector.memset(ONES[:], 1.0)
            nc.vector.tensor_copy(out=SUM2[:, 0:1], in_=ILS[:])
            nc.vector.tensor_copy(out=SUM2[:, 1:2], in_=CSUM[:])
            with tc.tile_pool(name="fpsum", bufs=1, space="PSUM") as fpsum:
                SP = fpsum.tile([1, 2], DT.float32)
                nc.tensor.matmul(SP[:], ONES[:], SUM2[:], start=True, stop=True)
                FIN = pers.tile([1, 2], DT.float32)
                nc.vector.tensor_copy(out=FIN[:], in_=SP[:])

            # ---- 8-byte AllReduce of the two partials ----------------------
            cc_in = dram.tile([1, 2], DT.float32)
            cc_out = dram.tile([1, 2], DT.float32, addr_space="Shared")
            nc.sync.dma_start(cc_in[:], FIN[:])
            nc.gpsimd.collective_compute(
                "AllReduce",
                OP.add,
                replica_groups=[list(range(ncores))],
                ins=[cc_in[:].opt()],
                outs=[cc_out[:].opt()],
            )
            TOT = pers.tile([1, 2], DT.float32)
            nc.sync.dma_start(TOT[:], cc_out[:])

            A = pers.tile([1, 1], DT.float32)
            B = pers.tile([1, 1], DT.float32)
            OUTS = pers.tile([1, 1], DT.float32)
            nc.vector.tensor_scalar(out=A[:], in0=TOT[0:1, 0:1],
                                    scalar1=1.0 / (n_pred * 3), scalar2=None, op0=OP.mult)
            nc.vector.tensor_scalar(out=B[:], in0=TOT[0:1, 1:2],
                                    scalar1=1.0 / n_pred, scalar2=None, op0=OP.mult)
            nc.vector.tensor_tensor(out=OUTS[:], in0=A[:], in1=B[:], op=OP.subtract)
            nc.vector.tensor_scalar(out=OUTS[:], in0=OUTS[:], scalar1=1.0,
                                    scalar2=None, op0=OP.add)
            nc.sync.dma_start(out_d[:], OUTS[:])

    nc.compile()
    return nc


# ----------------------------------------------------------------------------
# public entry point
# ----------------------------------------------------------------------------

_CACHED_NC = None


def kernel(pred_feat, pred_decoder, input_data, gt_data):
    global _CACHED_NC
    from concourse.bass_utils import run_bass_kernel_spmd

    in_maps = prep_inputs(pred_feat, gt_data, NCORES)
    if _CACHED_NC is None:
        _CACHED_NC = build_nc(N_PRED, L_GT, NCORES)
    res = run_bass_kernel_spmd(_CACHED_NC, in_maps, list(range(NCORES)),
                               trace=bool(int(os.environ.get("KERNEL_TRACE", "0"))))
    out = np.asarray(res.results[0]["out"], np.float32).reshape(())
    kernel.last_results = res
    return out


# revision 9
# speedup vs baseline: 1.2185x; 1.2185x over previous
"""Trainium2 Bass kernel for nn_CombinedCriterionAE (retrieval 1-NN + losses).

V2 strategy (8 NeuronCores, SPMD, pred-sharded):
  - preds are sharded across cores (1024/core, 8 tiles of 128); every core
    holds the FULL gt set (32768).  Each core computes exact argmin over the
    full row, so no cross-core argmin machinery is needed — the only
    collective is an 8-byte AllReduce of the two loss partial sums.  This
    removes the baseline's 250us tail (AllGather + 8-way fold + 128
    serialized indirect gathers).
  - s = -dist^2 = 2 p.g - p^2 - g^2 on the PE as a K=24 bf16 matmul
    (fp32 operands split host-side into 3 exact bf16 terms; small
    correction rows first, big rows last) -- s matches the reference's
    fp32 rounding to ~1e-6 so argmin picks track the reference.
  - Per 2048-col PSUM chunk: ACT stages the upper 1024 cols to SBUF (DVE
    cannot read two PSUM operands), one DVE tensor_tensor_scan computes the
    running max of pairs (j, j+1024), chained across the 8 chunks of a
    16384-wide super-chunk.
  - Per super-chunk (2 per tile): ONE ACT sign+accum over the full 8192
    scan outputs against the super-chunk's own max gives the first-occurrence
    winner position; the supermax lands in CMALL, the count in CNTALL.
    Counting against the LOCAL supermax (not a global max) keeps scan
    buffers short-lived and lets the sign pass batch 8 chunks into one
    instruction (amortizes the accumulator-read).
  - After the loop, positions resolve vectorized over all [128, 8, 2]
    candidates at once (no data-dependent addressing): winner super-chunk by
    is_ge (ties -> earlier), pair member by gathering both candidate gt rows
    (16 indirect DMAs) and comparing fp32 dist^2.
"""
import os
import numpy as np
import ml_dtypes

import concourse.bass as bass
import concourse.bacc as bacc
import concourse.mybir as mybir
import concourse.tile as tile
from concourse.bass import IndirectOffsetOnAxis

BF16 = ml_dtypes.bfloat16
DT = mybir.dt
OP = mybir.AluOpType

N_PRED = 8192
L_GT = 32768
NCORES = 8
K_SMALL = 19
K_BIG = 5
NEG_INF = -3.0e38


# ----------------------------------------------------------------------------
# host-side input prep
# ----------------------------------------------------------------------------

def _split3(x):
    x = np.asarray(x, np.float32)
    hi = x.astype(BF16)
    r = x - hi.astype(np.float32)
    mid = r.astype(BF16)
    r2 = r - mid.astype(np.float32)
    lo = r2.astype(BF16)
    return hi, mid, lo


def build_operands(pred_pts, gt_pts):
    """lhsT [24, N] / rhs [24, L] bf16; 19 small rows then 5 big rows."""
    q = 2.0 * np.asarray(pred_pts, np.float32)
    qh, qm, ql = _split3(q.T)
    gh, gm, gl = _split3(np.asarray(gt_pts, np.float32).T)
    g2 = (np.asarray(gt_pts, np.float32) ** 2).sum(1)
    p2 = (np.asarray(pred_pts, np.float32) ** 2).sum(1)
    g2h, g2m, g2l = _split3(g2)
    p2h, p2m, p2l = _split3(p2)
    ones_g = np.ones(gt_pts.shape[0], BF16)
    neg1_p = -np.ones(pred_pts.shape[0], BF16)

    lhs, rhs = [], []

    def add(a, b):
        lhs.append(a)
        rhs.append(b)

    for d in range(3):
        add(qh[d], gm[d]); add(qm[d], gh[d]); add(qm[d], gm[d])
        add(qh[d], gl[d]); add(ql[d], gh[d])
    add(neg1_p, g2m); add(neg1_p, g2l)
    add((-p2m).astype(BF16), ones_g); add((-p2l).astype(BF16), ones_g)
    # big rows
    add(qh[0], gh[0]); add(qh[1], gh[1]); add(qh[2], gh[2])
    add((-p2h).astype(BF16), ones_g); add(neg1_p, g2h)
    return np.ascontiguousarray(np.stack(lhs)), np.ascontiguousarray(np.stack(rhs))


def prep_inputs(pred_feat, gt_data, ncores):
    """Returns the per-core in_map list (preds sharded, gt full)."""
    pred_feat = np.asarray(pred_feat, np.float32)
    gt_data = np.asarray(gt_data, np.float32)
    n_pred = pred_feat.shape[0]
    npc = n_pred // ncores          # preds per core
    nt = npc // 128                 # pred tiles per core
    pred_pts = pred_feat[:, :3]
    pred_nrm = pred_feat[:, 3:]
    lhsT, rhs = build_operands(pred_pts, gt_data[:, :3])

    in_maps = []
    for c in range(ncores):
        sl = slice(npc * c, npc * (c + 1))
        pp = np.ascontiguousarray(
            pred_pts[sl].reshape(nt, 128, 3).transpose(1, 0, 2))
        pn = np.ascontiguousarray(
            pred_nrm[sl].reshape(nt, 128, 3).transpose(1, 0, 2))
        in_maps.append({
            "lhs": np.ascontiguousarray(lhsT[:, sl]),
            "rhs": rhs,
            "pp": pp,
            "pn": pn,
            "gtf": gt_data,
        })
    return in_maps


# ----------------------------------------------------------------------------
# device program
# ----------------------------------------------------------------------------

def build_nc(n_pred=N_PRED, l_gt=L_GT, ncores=NCORES):
    npc = n_pred // ncores
    nt = npc // 128                 # 8 pred tiles per core
    nsc = 2                         # super-chunks per tile
    nch = l_gt // (nsc * 2048)      # 8 chunks per super-chunk
    sw = nch * 1024                 # scan width per super-chunk (8192)
    assert nch * nsc * 2048 == l_gt

    nc = bacc.Bacc("TRN2", target_bir_lowering=False, debug=False,
                   num_devices=ncores)

    kk = K_SMALL + K_BIG
    lhs_d = nc.dram_tensor("lhs", [kk, npc], DT.bfloat16, kind="ExternalInput")
    rhs_d = nc.dram_tensor("rhs", [kk, l_gt], DT.bfloat16, kind="ExternalInput")
    pp_d = nc.dram_tensor("pp", [128, nt, 3], DT.float32, kind="ExternalInput")
    pn_d = nc.dram_tensor("pn", [128, nt, 3], DT.float32, kind="ExternalInput")
    gtf_d = nc.dram_tensor("gtf", [l_gt, 6], DT.float32, kind="ExternalInput")
    out_d = nc.dram_tensor("out", [1, 1], DT.float32, kind="ExternalOutput")

    with tile.TileContext(nc) as tc:
        with (
            tc.tile_pool(name="persist", bufs=1) as pers,
            tc.tile_pool(name="hpool", bufs=3) as hpool,
            tc.tile_pool(name="scnp", bufs=2) as scnp,
            tc.tile_pool(name="mkp", bufs=2) as mkp,
            tc.tile_pool(name="dram", bufs=1, space="DRAM") as dram,
        ):
            # ---- persistent SBUF loads -------------------------------------
            LHS = pers.tile([kk, npc], DT.bfloat16)
            RHS = pers.tile([kk, l_gt], DT.bfloat16)
            PP = pers.tile([128, nt, 3], DT.float32)
            PN = pers.tile([128, nt, 3], DT.float32)
            nc.sync.dma_start(LHS[:], lhs_d[:])
            # split the RHS load so the first matmuls only wait on slice 0
            for ksl in range(16):
                sl = slice(2048 * ksl, 2048 * (ksl + 1))
                nc.sync.dma_start(RHS[:, sl], rhs_d[:, sl])
            nc.sync.dma_start(PP[:], pp_d[:])
            nc.sync.dma_start(PN[:], pn_d[:])

            # per-(tile, super-chunk) results: columns s*nt+i; counts come in
            # 4 pieces of 2048 per super-chunk.  A full piece counts 2048
            # (prefix below the max throughout), the winner piece counts the
            # in-piece offset, later pieces count 0 -- so sum(pieces) = p and
            # the count of full pieces recovers the winner chunk.
            npieces = 4
            pw = sw // npieces
            CMALL = pers.tile([128, nsc * nt], DT.float32)
            CNT8 = pers.tile([128, nsc * nt, npieces], DT.float32)

            # ---- main loop: s-matmul, ACT staging, pairwise-max scan -------
            # The sign+count of super-chunk k is split into 2048-wide pieces
            # emitted between the NEXT super-chunk's stage copies, so ACT's
            # in-order queue never delays a stage copy for long (a single
            # 8192-wide sign blob would stall the scans behind the staging).
            def emit_piece(pend, j):
                SCNp, smax_p, kp = pend
                MK = mkp.tile([128, pw], DT.float16, tag="MK")
                nc.scalar.activation(
                    out=MK[:], in_=SCNp[:, pw * j:pw * (j + 1)],
                    func=mybir.ActivationFunctionType.Sign,
                    bias=smax_p, scale=-1.0,
                    accum_out=CNT8[:, kp, j:j + 1],
                )

            pending = None
            with tc.tile_pool(name="spsum", bufs=2, space="PSUM") as spsum:
                for i in range(nt):
                    for s in range(nsc):
                        SCN = scnp.tile([128, sw], DT.float32, tag="SCN")
                        for c in range(nch):
                            P = spsum.tile([128, 2048], DT.float32, tag="P")
                            for t in range(4):
                                col = 2048 * (nch * s + c) + 512 * t
                                nc.tensor.matmul(
                                    P[:, 512 * t:512 * (t + 1)],
                                    LHS[:, 128 * i:128 * (i + 1)],
                                    RHS[:, col:col + 512],
                                    start=True, stop=True,
                                )
                            HB = hpool.tile([128, 1024], DT.float32, tag="HB")
                            nc.scalar.activation(
                                out=HB[:], in_=P[:, 1024:2048],
                                func=mybir.ActivationFunctionType.Copy,
                            )
                            nc.vector.tensor_tensor_scan(
                                out=SCN[:, 1024 * c:1024 * (c + 1)],
                                data0=P[:, 0:1024],
                                data1=HB[:],
                                initial=(NEG_INF if c == 0
                                         else SCN[:, 1024 * c - 1:1024 * c]),
                                op0=OP.max,
                                op1=OP.max,
                            )
                            if pending is not None and c % 2 == 1:
                                emit_piece(pending, c // 2)
                        k = s * nt + i
                        smax_ap = SCN[:, sw - 1:sw]
                        nc.vector.tensor_copy(out=CMALL[:, k:k + 1], in_=smax_ap)
                        pending = (SCN, smax_ap, k)
                # last super-chunk's pieces
                for j in range(npieces):
                    emit_piece(pending, j)

            # counts -> p = sum(pieces); F = count of full (2048) pieces.
            # col_lo = 16384*s + 2048*c* + o* with p = 1024*c* + o* works out
            # to p + 2048*F + 1024*[p - 2048*F >= 1024].
            CNTALL = pers.tile([128, nsc * nt], DT.float32)
            nc.vector.tensor_reduce(out=CNTALL[:], in_=CNT8[:],
                                    axis=mybir.AxisListType.X, op=OP.add)
            GEQ = pers.tile([128, nsc * nt, npieces], DT.float32)
            nc.vector.tensor_scalar(out=GEQ[:], in0=CNT8[:],
                                    scalar1=float(pw) - 0.5, scalar2=float(pw),
                                    op0=OP.is_ge, op1=OP.mult)
            CQ = pers.tile([128, nsc * nt], DT.float32)
            nc.vector.tensor_reduce(out=CQ[:], in_=GEQ[:],
                                    axis=mybir.AxisListType.X, op=OP.add)
            COL16 = pers.tile([128, nsc * nt], DT.float32)
            WW = pers.tile([128, nsc * nt], DT.float32)
            nc.vector.tensor_tensor(out=WW[:], in0=CNTALL[:], in1=CQ[:], op=OP.subtract)
            nc.vector.tensor_scalar(out=WW[:], in0=WW[:],
                                    scalar1=1023.5, scalar2=1024.0,
                                    op0=OP.is_ge, op1=OP.mult)
            SBASE = pers.tile([128, nsc * nt], DT.float32)
            nc.vector.memset(SBASE[:, 0:nt], 0.0)
            nc.vector.memset(SBASE[:, nt:2 * nt], float(sw * 2))
            nc.vector.tensor_tensor(out=COL16[:], in0=CNTALL[:], in1=CQ[:], op=OP.add)
            nc.vector.tensor_tensor(out=COL16[:], in0=COL16[:], in1=WW[:], op=OP.add)
            nc.vector.tensor_tensor(out=COL16[:], in0=COL16[:], in1=SBASE[:], op=OP.add)

            # ---- pick the winning super-chunk per pred ---------------------
            CM0 = CMALL[:, 0:nt]
            CM1 = CMALL[:, nt:2 * nt]
            W0 = pers.tile([128, nt], DT.uint8)
            nc.vector.tensor_tensor(out=W0[:], in0=CM0, in1=CM1, op=OP.is_ge)
            L0G = pers.tile([128, nt], DT.float32)
            nc.vector.select(out=L0G[:], mask=W0[:], on_true=COL16[:, 0:nt],
                             on_false=COL16[:, nt:2 * nt])
            L1G = pers.tile([128, nt], DT.float32)
            nc.vector.tensor_scalar(out=L1G[:], in0=L0G[:], scalar1=1024.0,
                                    scalar2=None, op0=OP.add)

            # ---- gather both candidates, resolve the pair member -----------
            I0 = pers.tile([128, nt], DT.int32)
            I1 = pers.tile([128, nt], DT.int32)
            nc.vector.tensor_copy(out=I0[:], in_=L0G[:])
            nc.vector.tensor_copy(out=I1[:], in_=L1G[:])
            G0 = pers.tile([128, nt, 6], DT.float32)
            G1 = pers.tile([128, nt, 6], DT.float32)
            # HW supports one offset per partition per indirect DMA, so
            # gather tile-by-tile.
            for i in range(nt):
                nc.gpsimd.indirect_dma_start(
                    out=G0[:, i, :], out_offset=None, in_=gtf_d[:],
                    in_offset=IndirectOffsetOnAxis(ap=I0[:, i:i + 1], axis=0),
                )
                nc.gpsimd.indirect_dma_start(
                    out=G1[:, i, :], out_offset=None, in_=gtf_d[:],
                    in_offset=IndirectOffsetOnAxis(ap=I1[:, i:i + 1], axis=0),
                )
            DF = pers.tile([128, nt, 3], DT.float32)
            SQ = pers.tile([128, nt, 3], DT.float32)
            D0 = pers.tile([128, nt], DT.float32)
            D1 = pers.tile([128, nt], DT.float32)
            nc.vector.tensor_tensor(out=DF[:], in0=PP[:], in1=G0[:, :, 0:3], op=OP.subtract)
            nc.vector.tensor_tensor(out=SQ[:], in0=DF[:], in1=DF[:], op=OP.mult)
            nc.vector.tensor_reduce(out=D0[:], in_=SQ[:], axis=mybir.AxisListType.X, op=OP.add)
            nc.vector.tensor_tensor(out=DF[:], in0=PP[:], in1=G1[:, :, 0:3], op=OP.subtract)
            nc.vector.tensor_tensor(out=SQ[:], in0=DF[:], in1=DF[:], op=OP.mult)
            nc.vector.tensor_reduce(out=D1[:], in_=SQ[:], axis=mybir.AxisListType.X, op=OP.add)
            MEM = pers.tile([128, nt], DT.uint8)
            nc.vector.tensor_tensor(out=MEM[:], in0=D1[:], in1=D0[:], op=OP.is_ge)
            MATCH = pers.tile([128, nt, 6], DT.float32)
            for d in range(6):
                nc.vector.select(out=MATCH[:, :, d], mask=MEM[:],
                                 on_true=G0[:, :, d], on_false=G1[:, :, d])

            # ---- losses (partial sums over this core's preds) --------------
            ILS = pers.tile([128, 1], DT.float32)
            JNK = pers.tile([128, nt, 3], DT.float32)
            nc.vector.tensor_tensor(out=DF[:], in0=PP[:], in1=MATCH[:, :, 0:3], op=OP.subtract)
            nc.vector.tensor_tensor(out=JNK[:], in0=DF[:], in1=DF[:], op=OP.mult)
            nc.vector.tensor_reduce(out=ILS[:], in_=JNK[:],
                                    axis=mybir.AxisListType.XY, op=OP.add)

            def normalize(src3, dst3, tagp):
                NSQ = pers.tile([128, nt, 3], DT.float32, tag=f"NSQ{tagp}", name=f"NSQ{tagp}")
                NS = pers.tile([128, nt], DT.float32, tag=f"NS{tagp}", name=f"NS{tagp}")
                nc.vector.tensor_tensor(out=NSQ[:], in0=src3, in1=src3, op=OP.mult)
                nc.vector.tensor_reduce(out=NS[:], in_=NSQ[:], axis=mybir.AxisListType.X, op=OP.add)
                nc.scalar.activation(out=NS[:], in_=NS[:], func=mybir.ActivationFunctionType.Sqrt)
                nc.vector.tensor_scalar(out=NS[:], in0=NS[:], scalar1=1e-4,
                                        scalar2=None, op0=OP.max)
                nc.vector.reciprocal(out=NS[:], in_=NS[:])
                for d in range(3):
                    nc.vector.tensor_tensor(out=dst3[:, :, d], in0=src3[:, :, d],
                                            in1=NS[:], op=OP.mult)

            PNH = pers.tile([128, nt, 3], DT.float32)
            MNH = pers.tile([128, nt, 3], DT.float32)
            normalize(PN[:], PNH, "a")
            normalize(MATCH[:, :, 3:6], MNH, "b")
            CC3 = pers.tile([128, nt, 3], DT.float32)
            CSUM = pers.tile([128, 1], DT.float32)
            nc.vector.tensor_tensor(out=CC3[:], in0=PNH[:], in1=MNH[:], op=OP.mult)
            nc.vector.tensor_reduce(out=CSUM[:], in_=CC3[:],
                                    axis=mybir.AxisListType.XY, op=OP.add)

            # partition-sum via ones-matmul -> [1, 2] partials
            SUM2 = pers.tile([128, 2], DT.float32)
            ONES = pers.tile([128, 1], DT.float32)
            nc.vector.memset(ONES[:], 1.0)
            nc.vector.tensor_copy(out=SUM2[:, 0:1], in_=ILS[:])
            nc.vector.tensor_copy(out=SUM2[:, 1:2], in_=CSUM[:])
            with tc.tile_pool(name="fpsum", bufs=1, space="PSUM") as fpsum:
                SP = fpsum.tile([1, 2], DT.float32)
                nc.tensor.matmul(SP[:], ONES[:], SUM2[:], start=True, stop=True)
                FIN = pers.tile([1, 2], DT.float32)
                nc.vector.tensor_copy(out=FIN[:], in_=SP[:])

            # ---- 8-byte AllReduce of the two partials ----------------------
            cc_in = dram.tile([1, 2], DT.float32)
            cc_out = dram.tile([1, 2], DT.float32, addr_space="Shared")
            nc.sync.dma_start(cc_in[:], FIN[:])
            nc.gpsimd.collective_compute(
                "AllReduce",
                OP.add,
                replica_groups=[list(range(ncores))],
                ins=[cc_in[:].opt()],
                outs=[cc_out[:].opt()],
            )
            TOT = pers.tile([1, 2], DT.float32)
            nc.sync.dma_start(TOT[:], cc_out[:])

            A = pers.tile([1, 1], DT.float32)
            B = pers.tile([1, 1], DT.float32)
            OUTS = pers.tile([1, 1], DT.float32)
            nc.vector.tensor_scalar(out=A[:], in0=TOT[0:1, 0:1],
                                    scalar1=1.0 / (n_pred * 3), scalar2=None, op0=OP.mult)
            nc.vector.tensor_scalar(out=B[:], in0=TOT[0:1, 1:2],
                                    scalar1=1.0 / n_pred, scalar2=None, op0=OP.mult)
            nc.vector.tensor_tensor(out=OUTS[:], in0=A[:], in1=B[:], op=OP.subtract)
            nc.vector.tensor_scalar(out=OUTS[:], in0=OUTS[:], scalar1=1.0,
                                    scalar2=None, op0=OP.add)
            nc.sync.dma_start(out_d[:], OUTS[:])

    nc.compile()
    return nc


# ----------------------------------------------------------------------------
# public entry point
# ----------------------------------------------------------------------------

_CACHED_NC = None


def kernel(pred_feat, pred_decoder, input_data, gt_data):
    global _CACHED_NC
    from concourse.bass_utils import run_bass_kernel_spmd

    in_maps = prep_inputs(pred_feat, gt_data, NCORES)
    if _CACHED_NC is None:
        _CACHED_NC = build_nc(N_PRED, L_GT, NCORES)
    res = run_bass_kernel_spmd(_CACHED_NC, in_maps, list(range(NCORES)),
                               trace=bool(int(os.environ.get("KERNEL_TRACE", "0"))))
    out = np.asarray(res.results[0]["out"], np.float32).reshape(())
    kernel.last_results = res
    return out
